# revision 51
# baseline (speedup 1.0000x reference)
"""MultiResolutionHashEncoding Trainium2 kernel.

Strategy (per NeuronCore, LEVEL-sharded: core c handles levels 2c, 2c+1 for
the full batch, so each core loads its tables exactly once — no per-level
reload bubbles; resolutions are per-core data to keep the program SPMD):
  - Hash indices are computed on DVE with exact int math in the fp32-safe
    range (products kept < 2^24, primes pre-reduced mod 2^19).
  - The 2^19-entry tables are sliced into 16 slices of 2^15 entries; each
    GpSimd core's 16 partitions hold one full table copy (slice s on
    partition 16c+s) as scaled fp16 pairs.
  - ap_gather (per-core shared int16 index stream = low 15 bits) fetches the
    16 candidate entries per element.
  - The slice-select mask is computed without a second gather: 16 selector
    matmuls on the (idle) PE broadcast each element's high-4 hash bits to
    all 16 partitions of its group, and DVE is_equal against the partition's
    slice id yields the {0,1} fp16 mask.
  - DVE multiplies candidates by the mask; PE contracts each 16-partition
    group with a block-diagonal ones matrix; the selected values are DMA'd
    out in a strided pattern.
"""

import numpy as np

import concourse.tile_utils as tile_utils

tile_utils.max_sbuf_usage = 206 * 1024  # stale 192K default; cayman has 208K usable

import concourse.bacc as bacc
import concourse.tile as tile
import concourse.mybir as mybir
from concourse import bass
from concourse.bass_utils import run_bass_kernel_spmd

AluOp = mybir.AluOpType
dt = mybir.dt

N_LEVELS = 16
N_FEATS = 2
TABLE_SIZE = 524288  # 2**19
RESOLUTIONS = [16, 23, 32, 45, 64, 91, 128, 181, 256, 362, 512, 724, 1024,
               1448, 2048, 2896]
PRIMES = (1, 2654435761, 805459861)
BATCH = 2_097_152
N_CORES = 8

P = 128
L_PER_CORE = N_LEVELS // N_CORES  # 2 levels per core (level-sharded)
SPP = BATCH // P               # 16384 elements per partition (full batch)
S_CHUNK = 256                  # s-range per processing chunk
N_CHUNKS = SPP // S_CHUNK      # 64 chunks per level
NI = 16 * S_CHUNK              # ap_gather num_idxs per core per chunk (4096)
SLICE = 32768                  # table entries per partition slice
SCALE = float(2 ** 13)         # table scaling for fp16 storage
MASK19 = 0x7FFFF

K1 = PRIMES[1] & MASK19        # 489905
K2 = PRIMES[2] & MASK19        # 95765

LAST_EXEC_SECONDS = None
LAST_EXEC_NS = None
LAST_TRACE = None


def _emit_floor(nc, pool, psumh, src, r_ap, out_dtype, tag, S,
                g_in_psum=False):
    """g = floor(src * R) for src f32 [P, S]; exact w.r.t. f32 product.

    R comes in as a [P, 1] f32 AP (per-core data, since levels are sharded
    across cores but the program is SPMD).  Single-input ops run on ACT
    (own SBUF port), and the fp32 intermediates live in PSUM, so the DVE
    compare/subtract ops don't steal the DVE/GpSimd shared SBUF port from
    the ap_gather ucode.  g_in_psum additionally parks the result in PSUM
    (valid when the only consumer is ACT, i.e. the fp32 gy/gz).
    """
    v = psumh.tile([P, S], dt.float32, space="PSUM", tag="fl_v")
    nc.scalar.mul(v[:], src[:], r_ap)
    # ACT cast: any monotone rounding gives r_i in {floor, floor+1}, and the
    # c/g correction below yields exact floor for either
    r_i = pool.tile([P, S], dt.int32, tag="fl_ri")
    nc.scalar.copy(r_i[:], v[:])
    r_f = pool.tile([P, S], dt.float32, tag="fl_rf")
    nc.scalar.copy(r_f[:], r_i[:])
    # only one DVE operand may come from PSUM; v is the PSUM one
    c = psumh.tile([P, S], dt.float32, space="PSUM", tag="fl_c")
    nc.vector.tensor_tensor(c[:], v[:], r_f[:], AluOp.is_lt)  # v < r_f -> 1.0
    if g_in_psum:
        g = psumh.tile([P, S], out_dtype, space="PSUM", tag=tag)
    else:
        g = pool.tile([P, S], out_dtype, tag=tag)
    nc.vector.tensor_tensor(g[:], r_f[:], c[:], AluOp.subtract)
    return g


def _emit_prime_mul(nc, pool, g_f, K, tag, S):
    """int32 tile whose low 19 bits equal (g*K) mod 2^19 (g < 4096)."""
    Khi, Klo = K >> 7, K & 127
    a = pool.tile([P, S], dt.int32, tag="pm_a")
    nc.scalar.mul(a[:], g_f[:], float(Khi))      # exact int product, ACT
    b = pool.tile([P, S], dt.int32, tag="pm_b")
    nc.scalar.mul(b[:], g_f[:], float(Klo))
    a0 = pool.tile([P, S], dt.int32, tag="pm_a0")
    nc.vector.tensor_scalar(a0[:], a[:], 0xFFF, None, AluOp.bitwise_and)
    comb = pool.tile([P, S], dt.int32, tag=tag)
    nc.vector.scalar_tensor_tensor(comb[:], a0[:], 128.0, b[:], AluOp.mult,
                                   AluOp.add)
    return comb


def build_nc():
    nc = bacc.Bacc(None, target_bir_lowering=False)

    # Per-core inputs: FULL coords [3, P, SPP] f32, this core's 2 levels of
    # replicated tables [L_PER_CORE, P, SLICE, 2] f16 (pre-scaled by SCALE),
    # the 2 resolutions as data, block-diag ones + selector weights.
    coords_in = nc.dram_tensor("coords3", [3, P, SPP], dt.float32,
                               kind="ExternalInput")
    tblr = nc.dram_tensor("tblr", [L_PER_CORE, P, SLICE, N_FEATS],
                          dt.float16, kind="ExternalInput")
    rt_in = nc.dram_tensor("rt", [P, L_PER_CORE], dt.float32,
                           kind="ExternalInput")
    b16_in = nc.dram_tensor("b16", [P, 8], dt.float16, kind="ExternalInput")
    wq_in = nc.dram_tensor("wq", [P, 16, P], dt.float16,
                           kind="ExternalInput")
    sid_in = nc.dram_tensor("sid", [P, 1], dt.float32, kind="ExternalInput")
    out = nc.dram_tensor("out", [L_PER_CORE, N_CHUNKS, 8, NI * N_FEATS],
                         dt.float32, kind="ExternalOutput")

    with tile.TileContext(nc) as tc:
        with (
            tc.tile_pool(name="tabp", bufs=1) as tabp,
            tc.tile_pool(name="workp", bufs=2) as workp,
            tc.tile_pool(name="hashp", bufs=1) as hashp,
            tc.tile_pool(name="iop", bufs=2) as iop,
            tc.tile_pool(name="coop", bufs=2) as coop,
            tc.tile_pool(name="selp", bufs=4) as selp,
            tc.tile_pool(name="constp", bufs=1) as constp,
            tc.tile_pool(name="psump", bufs=2, space="PSUM") as psump,
            tc.tile_pool(name="psumq", bufs=1, space="PSUM") as psumq,
            tc.tile_pool(name="psumh", bufs=1, space="PSUM") as psumh,
        ):
            b16 = constp.tile([P, 8], dt.float16, tag="b16")
            nc.sync.dma_start(b16[:], b16_in[:])
            wq = constp.tile([P, 16, P], dt.float16, tag="wq")
            nc.sync.dma_start(wq[:], wq_in[:])
            sid = constp.tile([P, 1], dt.float32, tag="sid")
            nc.sync.dma_start(sid[:], sid_in[:])
            rt = constp.tile([P, L_PER_CORE], dt.float32, tag="rt")
            nc.sync.dma_start(rt[:], rt_in[:])
            mask19t = constp.tile([P, 1], dt.int32, tag="mask19t")
            nc.vector.memset(mask19t[:], MASK19)

            tabt = tabp.tile([P, SLICE, N_FEATS], dt.float16, tag="tabt")

            def load_coords(ch):
                s0 = ch * S_CHUNK
                sl = slice(s0, s0 + S_CHUNK)
                xt = coop.tile([P, S_CHUNK], dt.float32, tag="xt")
                yt = coop.tile([P, S_CHUNK], dt.float32, tag="yt")
                zt = coop.tile([P, S_CHUNK], dt.float32, tag="zt")
                nc.sync.dma_start(xt[:], coords_in[0, :, sl])
                nc.sync.dma_start(yt[:], coords_in[1, :, sl])
                nc.sync.dma_start(zt[:], coords_in[2, :, sl])
                return xt, yt, zt

            def emit_front(lvl, ch, pre=None):
                """coords load + hash + candidate-gather issue."""
                r_ap = rt[:, lvl:lvl + 1]
                xt, yt, zt = pre if pre is not None else load_coords(ch)

                gx = _emit_floor(nc, hashp, psumh, xt, r_ap, dt.int32, "gx",
                                 S_CHUNK)
                gy = _emit_floor(nc, hashp, psumh, yt, r_ap, dt.float32,
                                 "gy", S_CHUNK, g_in_psum=True)
                gz = _emit_floor(nc, hashp, psumh, zt, r_ap, dt.float32,
                                 "gz", S_CHUNK, g_in_psum=True)
                py_ = _emit_prime_mul(nc, hashp, gy, K1, "py", S_CHUNK)
                pz_ = _emit_prime_mul(nc, hashp, gz, K2, "pz", S_CHUNK)
                # scratch tags pm_a/pm_b/pm_a0 are dead by now; alias them
                t1 = hashp.tile([P, S_CHUNK], dt.int32, tag="pm_a")
                nc.vector.scalar_tensor_tensor(
                    t1[:], py_[:], mask19t[:], gx[:],
                    AluOp.bitwise_and, AluOp.bitwise_xor)
                h = hashp.tile([P, S_CHUNK], dt.int32, tag="pm_b")
                nc.vector.scalar_tensor_tensor(
                    h[:], pz_[:], mask19t[:], t1[:],
                    AluOp.bitwise_and, AluOp.bitwise_xor)
                lo32 = hashp.tile([P, S_CHUNK], dt.int32, tag="pm_a0")
                nc.vector.tensor_scalar(lo32[:], h[:], 0x7FFF, None,
                                        AluOp.bitwise_and)
                lo = iop.tile([P, S_CHUNK], dt.int16, tag="lo")
                nc.scalar.copy(lo[:], lo32[:])       # exact narrowing, ACT
                hi32 = hashp.tile([P, S_CHUNK], dt.int32, tag="pm_a")
                nc.vector.tensor_scalar(hi32[:], h[:], 15, None,
                                        AluOp.logical_shift_right)
                hi_f = iop.tile([P, S_CHUNK], dt.float16, tag="hi_f")
                nc.scalar.copy(hi_f[:], hi32[:])     # exact cast, ACT

                cand = workp.tile([P, NI, N_FEATS], dt.float16, tag="cand")
                nc.gpsimd.ap_gather(cand[:], tabt[:], lo[:], channels=P,
                                    num_elems=SLICE, d=N_FEATS,
                                    num_idxs=NI)
                return lvl, ch, cand, hi_f

            def emit_back(state):
                """slice-mask select + block-sum + store for a front chunk."""
                lvl, ch, cand, hi_f = state
                # broadcast hi across each 16-partition group via selector
                # matmuls on PE, then fused (hi==slice_id)*cand in place;
                # 4 q-lanes share one PSUM tile + one DVE op (fewer
                # dispatches stealing the DVE/GpSimd shared SBUF port)
                for qb in range(0, 16, 4):
                    psq4 = psumq.tile([P, 4, S_CHUNK], dt.float32,
                                      space="PSUM", tag="psq4")
                    for qi in range(4):
                        nc.tensor.matmul(psq4[:, qi, :], wq[:, qb + qi],
                                         hi_f[:], start=True, stop=True)
                    # STT is limited to 3-D APs: one op per feature column,
                    # each covering 4 q-lanes [P, s, q(4)]
                    pv = psq4[:].rearrange("p q s -> p s q")
                    cq = cand[:].rearrange(
                        "p (s q) f -> p s q f", q=16)
                    for f in range(N_FEATS):
                        cv = cq[:, :, qb:qb + 4, f]
                        nc.vector.scalar_tensor_tensor(
                            cv, pv, sid[:], cv, AluOp.is_equal, AluOp.mult)

                # block-sum on PE, descale on evacuation
                cfl = cand[:].rearrange("p n f -> p (n f)")
                NCOL = 512
                for mcol in range(0, NI * N_FEATS, NCOL):
                    ps = psump.tile([8, NCOL], dt.float32, space="PSUM",
                                    tag="ps")
                    nc.tensor.matmul(ps[:], b16[:],
                                     cfl[:, mcol:mcol + NCOL],
                                     start=True, stop=True)
                    sel = selp.tile([8, NCOL], dt.float32, tag="sel")
                    nc.scalar.mul(sel[:], ps[:], 1.0 / SCALE)
                    nc.sync.dma_start(out[lvl, ch, :, mcol:mcol + NCOL],
                                      sel[:])

            # one-stage software pipeline: next chunk's hash+gather is emitted
            # (and thus sequenced on DVE/Pool) ahead of the current chunk's
            # select stage, so gathers run back-to-back
            # chunk 0's coords go on the sync queue BEFORE the 16MB table
            # load so the first hash overlaps it
            pre0 = load_coords(0)
            pending = None
            for lvl in range(L_PER_CORE):
                nc.sync.dma_start(tabt[:], tblr[lvl])
                for ch in range(N_CHUNKS):
                    front = emit_front(lvl, ch,
                                       pre=pre0 if (lvl == 0 and ch == 0)
                                       else None)
                    if pending is not None:
                        emit_back(pending)
                    pending = front
            emit_back(pending)

    nc.compile()
    return nc


def _prep_tables(tables):
    """[L, T, F] f32 -> replicated sliced fp16 [L, P, SLICE, F] (scaled)."""
    t16 = (tables * SCALE).astype(np.float16)  # [L, T, F]
    sl = t16.reshape(N_LEVELS, 16, SLICE, N_FEATS)  # slice s = entries s*SLICE+
    # partition p holds slice p % 16
    return np.ascontiguousarray(sl[:, np.arange(P) % 16])  # [L, P, SLICE, F]


def kernel(coords, tables):
    global LAST_EXEC_SECONDS
    coords = np.asarray(coords, dtype=np.float32)
    tables = np.asarray(tables, dtype=np.float32)

    tblr = _prep_tables(tables)
    b16 = np.zeros((P, 8), np.float16)
    for g in range(8):
        b16[g * 16:(g + 1) * 16, g] = 1.0
    # selector weights: wq[p, q, c] = 1 iff p == (c//16)*16 + q
    wq = np.zeros((P, 16, P), np.float16)
    for q in range(16):
        for c in range(P):
            wq[(c // 16) * 16 + q, q, c] = 1.0
    sid = (np.arange(P) % 16).astype(np.float32).reshape(P, 1)

    nc = build_nc()

    # every core sees the FULL batch; core c owns levels [2c, 2c+1]
    c3 = np.ascontiguousarray(coords.T.reshape(3, P, SPP))
    in_maps = []
    for c in range(N_CORES):
        lv = slice(c * L_PER_CORE, (c + 1) * L_PER_CORE)
        rt = np.broadcast_to(
            np.asarray(RESOLUTIONS[lv], np.float32), (P, L_PER_CORE))
        rt = np.ascontiguousarray(rt)
        in_maps.append({"coords3": c3, "tblr": tblr[lv], "rt": rt,
                        "b16": b16, "wq": wq, "sid": sid})

    import time
    global LAST_EXEC_NS, LAST_TRACE
    t0 = time.time()
    res = run_bass_kernel_spmd(nc, in_maps, core_ids=list(range(N_CORES)))
    LAST_EXEC_SECONDS = time.time() - t0
    LAST_EXEC_NS = getattr(res, "exec_time_ns", None)
    LAST_TRACE = getattr(res, "instructions_and_trace", None)

    out = np.empty((BATCH, N_LEVELS * N_FEATS), np.float32)
    for c in range(N_CORES):
        oc = res.results[c]["out"]  # [L_PER_CORE, NCH, 8, NI*F]
        oc = oc.reshape(L_PER_CORE, N_CHUNKS, 8, S_CHUNK, 16, N_FEATS)
        # axes (l, ch, g, sj, q, f) -> b = ((g*16+q)*SPP + ch*S_CHUNK + sj)
        oc = oc.transpose(2, 4, 1, 3, 0, 5).reshape(
            BATCH, L_PER_CORE * N_FEATS)
        out[:, c * L_PER_CORE * N_FEATS:(c + 1) * L_PER_CORE * N_FEATS] = oc
    return out

